# revision 7
# baseline (speedup 1.0000x reference)
"""DualHOILoss Trainium2 kernel, v2: candidate-pruned distance mins.

Strategy (8 NeuronCores, the 512 point-cells dealt across cores):
  - Host splits each batch's 4096 points into 32 spatial cells of 128
    (KD median splits).  For each cell it computes a provably sufficient
    vert candidate list: verts within min(U_cell, D_cut) of the cell's
    AABB, where U_cell is an upper bound on any cell point's nearest-vert
    distance (via a 64-cluster decomposition of the verts) and D_cut is
    where exp(-100 d^2) is negligible vs the 2e-2 tolerance.  This cuts
    the candidate columns from 778/cell to ~210 avg.  All 512 cells are
    dealt snake-wise by width across the 8 cores so the shared SPMD
    program's rank-max width profile has minimal padding.
  - Device computes d^2[point, cand] via K=5 bf16 matmuls into PSUM
    (coefficient vectors packed on host; matmul outputs chunked on the
    512-col grid so they never cross a PSUM bank).  Each cell is
    min-reduced by ONE DVE tensor_tensor_scan consuming two streams per
    cycle; per drain group either the scan reads the PSUM A-half
    directly while ACT copies only the B-half (P1, ACT-cheap), or ACT
    copies the whole group and the scan is all-SBUF (P2, DVE-cheap) --
    chosen per group to balance the two engines.  Each scan writes
    through a stride-0 (broadcast) output AP, so its final running-min
    write lands the cell min directly in the mins slab, which is DMA'd
    out in two chunks.
  - Host finishes both losses from host-side O(P) data: the choir loss
    directly from the packed selected-anchor distances, the contact
    loss from the [128, 64] device min slab (exp + mean).  All O(P*V)
    work stays on device; the host does only O(P+V) packing and O(P)
    finishing.

The program is compiled per width-profile (deterministic for a given
input) and cached; unseen inputs just trigger a recompile.
"""

import numpy as np

B, P, A, V = 16, 4096, 32, 778
NCORES = 8
BPC = B // NCORES
NCELL = 32          # cells per batch
NSLOT = 2 * NCELL   # slots per core
D_CUT = 0.14
GROUP_COLS = 1024   # psum f32 cols per drain group (2 banks)
BIGD = 1.0e30       # sentinel distance^2 for padding columns
INF = 3.0e38
DMA_FRAC = 0.84     # fraction of B-half copies done by the DMA engines

_CACHE = {}


# ----------------------------------------------------------------- host side

def _kd_split(pts, idx, ncell):
    if ncell == 1:
        return [idx]
    d = np.argmax(pts[idx].max(0) - pts[idx].min(0))
    order = idx[np.argsort(pts[idx][:, d], kind="stable")]
    h = len(order) // 2
    return _kd_split(pts, order[:h], ncell // 2) + _kd_split(
        pts, order[h:], ncell // 2)


def _candidates(xb, yb):
    """Per-cell (point_ids[128], cand vert ids) for one batch."""
    cells = _kd_split(xb, np.arange(P), NCELL)
    vc = _kd_split(yb, np.arange(V), 64)
    cen = np.stack([yb[c].mean(0) for c in vc])
    rad = np.array([np.linalg.norm(yb[c] - cen[i], axis=1).max()
                    for i, c in enumerate(vc)])
    d2c = np.linalg.norm(xb[:, None, :] - cen[None], axis=2)
    ubp = (d2c + rad[None]).min(1)          # per-point nearest-vert UB
    out = []
    for ci in cells:
        lo = xb[ci].min(0)
        hi = xb[ci].max(0)
        dmin = np.linalg.norm(
            np.maximum(np.maximum(lo - yb, yb - hi), 0), axis=1)
        u_box = np.linalg.norm(
            np.maximum(np.abs(lo - yb), np.abs(hi - yb)), axis=1).min()
        u = min(ubp[ci].max(), u_box, D_CUT)
        cand = np.where(dmin <= u + 1e-6)[0]
        out.append((ci, cand))
    return out


def _level_dp(widths):
    """Partition sorted-desc widths into segments padded to the segment
    max, trading padding columns against per-group fixed costs."""
    n = len(widths)
    group_tax = 350  # columns-equivalent of one drain group's fixed cost
    best = [None] * (n + 1)
    best[0] = (0.0, 0)
    for i in range(1, n + 1):
        for j in range(max(0, i - 24), i):
            wmax = widths[j]           # sorted desc: max of [j, i)
            ln = i - j
            kpg = max(1, GROUP_COLS // wmax)
            cost = (best[j][0] + wmax * ln
                    + group_tax * int(np.ceil(ln / kpg)))
            if best[i] is None or cost < best[i][0]:
                best[i] = (cost, j)
    # recover segmentation
    out = [0] * n
    i = n
    while i > 0:
        j = best[i][1]
        for t in range(j, i):
            out[t] = widths[j]
        i = j
    return out


def _plan_groups(widths):
    groups = []  # (w, k, slot0)
    j = 0
    while j < len(widths):
        w = widths[j]
        k = 1
        while (j + k < len(widths) and widths[j + k] == w
               and (k + 1) * w <= GROUP_COLS):
            k += 1
        groups.append((w, k, j))
        j += k
    return groups


def _layout(widths, groups):
    """Per-group contiguous coef layout: [rhs slots..., lhsT slots...] per
    group, so DMA prefixes cover whole groups."""
    rhs_off = [0] * len(widths)
    lhs_off = [0] * len(widths)
    goff = []
    off = 0
    for (w, k, j0) in groups:
        goff.append(off)
        for kk in range(k):
            rhs_off[j0 + kk] = off
            off += w
        for kk in range(k):
            lhs_off[j0 + kk] = off
            off += 128
    goff.append(off)
    return rhs_off, lhs_off, goff


def _pack(verts, anchors, choir, hand_contacts, bps_mean, bps_scalar,
          bps_basis):
    import ml_dtypes
    verts = np.asarray(verts, np.float32)
    anchors = np.asarray(anchors, np.float32)
    choir = np.asarray(choir, np.float32)
    hand_contacts = np.asarray(hand_contacts, np.float32)
    bps_mean = np.asarray(bps_mean, np.float32).reshape(B, 3)
    s = np.float32(np.asarray(bps_scalar).reshape(()))
    basis = np.asarray(bps_basis, np.float32).reshape(P, 3)

    u = basis[None] + choir[:, :, 1:4]                 # (B,P,3)
    x = u / s
    w = verts - bps_mean[:, None, :]                   # (B,V,3)
    uu2 = (u * u).sum(-1) / (s * s)                    # (B,P)

    idx = choir[:, :, 5].astype(np.int64)
    asel = np.take_along_axis(anchors, idx[:, :, None], axis=1)
    wsel = asel - bps_mean[:, None, :]
    rsel = np.maximum((wsel * wsel).sum(-1)
                      - (np.float32(2.0) / s) * (u * wsel).sum(-1) + uu2,
                      np.float32(1.0e-12))             # (B,P)

    rhs_all = np.empty((B, 5, V), np.float32)
    rhs_all[:, 0:3] = (w * (np.float32(-2.0) / s)).transpose(0, 2, 1)
    rhs_all[:, 3] = (w * w).sum(-1)
    rhs_all[:, 4] = 1.0
    dummy_col = np.array([0.0, 0.0, 0.0, BIGD, 1.0], np.float32)

    cells = [_candidates(x[b], w[b]) for b in range(B)]
    bwidth = []
    for b in range(B):
        tw = sum(max(64, int(np.ceil(max(len(c), 1) / 32) * 32))
                 for (_, c) in cells[b])
        bwidth.append(tw)

    # global cell->core assignment: deal all B*NCELL cells in width order,
    # snake-wise across cores, so per-rank widths are nearly equal and the
    # shared (rank-max) program profile has minimal padding
    all_cells = []
    for b in range(B):
        for ci, (pid, cand) in enumerate(cells[b]):
            wd = max(32, int(np.ceil(max(len(cand), 1) / 16) * 16))
            all_cells.append((wd, b, ci))
    all_cells.sort(key=lambda t: -t[0])
    core_slots = [[] for _ in range(NCORES)]
    for r, (wd, b, ci) in enumerate(all_cells):
        row, col = divmod(r, NCORES)
        c = col if row % 2 == 0 else NCORES - 1 - col
        core_slots[c].append((b, ci, wd))

    widths = [max(core_slots[c][j][2] for c in range(NCORES))
              for j in range(NSLOT)]
    widths = _level_dp(widths)
    # rotate the last (smallest) group's slots to the front: a tiny first
    # group primes the pipeline right after the first DMA chunk lands
    g0 = _plan_groups(widths)
    nlast = NSLOT - g0[-1][2]
    jcut = NSLOT - min(4, nlast)
    perm = list(range(jcut, NSLOT)) + list(range(0, jcut))
    widths = [widths[p] for p in perm]
    for c in range(NCORES):
        core_slots[c] = [core_slots[c][p] for p in perm]
    groups = _plan_groups(widths)
    rhs_off, lhs_off, goff = _layout(widths, groups)
    lc = goff[-1]
    spec = (tuple(widths), tuple(groups))

    in_maps = []
    metas = []
    for c in range(NCORES):
        coef = np.zeros((5, lc), np.float32)
        pids = np.empty((NSLOT, 128), np.int64)
        bidx = np.empty(NSLOT, np.int64)
        for j in range(NSLOT):
            b, ci, _ = core_slots[c][j]
            pid, cand = cells[b][ci]
            wd = widths[j]
            off = rhs_off[j]
            if len(cand) > 0:
                cp = np.concatenate(
                    [cand, np.repeat(cand[:1], wd - len(cand))])
                coef[:, off:off + wd] = rhs_all[b][:, cp]
                if len(cand) < wd:
                    coef[:, off + len(cand):off + wd] = dummy_col[:, None]
            else:
                coef[:, off:off + wd] = dummy_col[:, None]
            lt = lhs_off[j]
            coef[0:3, lt:lt + 128] = u[b][pid].T
            coef[3, lt:lt + 128] = 1.0
            coef[4, lt:lt + 128] = uu2[b][pid]
            pids[j] = pid
            bidx[j] = b
        in_maps.append({
            "coef": coef.astype(ml_dtypes.bfloat16),
        })
        metas.append((pids, bidx))
    return spec, in_maps, metas, (lc, rsel)


# --------------------------------------------------------------- device side

def _build_program(spec):
    import concourse.bacc as bacc
    import concourse.mybir as mybir
    from concourse import tile

    widths, groups = spec
    f32 = mybir.dt.float32
    bf16 = mybir.dt.bfloat16
    AF = mybir.ActivationFunctionType
    ALU = mybir.AluOpType

    total_w = sum(widths)
    rhs_off, lhs_off, goff = _layout(widths, list(groups))
    lc = goff[-1]
    ngroups = len(groups)

    # Per-group drain path: P1 (scan reads PSUM directly, ACT copies only
    # the B-half) vs P2 (ACT copies the whole group to SBUF, scan is
    # all-SBUF).  P1 is ACT-cheap, P2 is DVE-cheap; flip the smallest
    # groups to P2 until the two engines' modeled loads balance.
    p2_group = [False] * ngroups
    dve_ns = sum(0.521 * w * k + 125 * k for (w, k, _) in groups)
    act_ns = sum(0.475 * w * k + 143 for (w, k, _) in groups) + 1200
    for gi in range(ngroups - 1, -1, -1):
        w, k, _ = groups[gi]
        d_dve = -74 * k
        d_act = 0.475 * w * k
        if act_ns + d_act + 300 < dve_ns + d_dve:
            p2_group[gi] = True
            dve_ns += d_dve
            act_ns += d_act

    nc = bacc.Bacc(None, target_bir_lowering=False)
    coef_d = nc.dram_tensor("coef", [5, lc], bf16, kind="ExternalInput")
    mins_d = nc.dram_tensor("mins", [128, NSLOT], f32,
                            kind="ExternalOutput")

    with tile.TileContext(nc) as tc:
        with (
            tc.tile_pool(name="sb", bufs=1) as sb,
            tc.tile_pool(name="wk", bufs=3) as wk,
            tc.tile_pool(name="ps", bufs=4, space="PSUM") as ps,
        ):
            coef = sb.tile([5, lc], bf16, tag="coef", name="coef")
            # two prefix chunks over the per-group layout: the first covers
            # the prime group plus enough groups to bridge until the rest
            # of the slab lands (HWDGE serializes at ~625ns/DMA)
            cut1 = goff[min(7, ngroups)]
            nc.sync.dma_start(coef[:, 0:cut1], coef_d[:, 0:cut1])
            nc.sync.dma_start(coef[:, cut1:], coef_d[:, cut1:])

            # PE p-state warmup
            wtile = sb.tile([5, 512], bf16, tag="wtile", name="wtile")
            nc.vector.memset(wtile[:], 0.0)
            wps = ps.tile([128, GROUP_COLS], f32, tag="pg", name="wps")
            for _ in range(4):
                nc.tensor.matmul(wps[:, 0:512], wtile[:, 0:128], wtile[:],
                                 start=True, stop=True)
            mins = sb.tile([128, NSLOT], f32, tag="mins", name="mins")

            slots_done = 0
            mins_sent = 0
            for gi, (w, k, j0) in enumerate(groups):
                pg = ps.tile([128, GROUP_COLS], f32, tag="pg", name=f"pg{gi}")
                for kk in range(k):
                    so = rhs_off[j0 + kk]
                    lt = lhs_off[j0 + kk]
                    o = kk * w
                    # chunk on the global 512-col grid: a matmul output
                    # must not cross a PSUM bank boundary
                    cc0 = 0
                    while cc0 < w:
                        nxt_bank = ((o + cc0) // 512 + 1) * 512 - o
                        cc1 = min(cc0 + 512, w, nxt_bank)
                        nc.tensor.matmul(
                            pg[:, o + cc0:o + cc1],
                            coef[:, lt:lt + 128],
                            coef[:, so + cc0:so + cc1],
                            start=True, stop=True)
                        cc0 = cc1
                h = w // 2
                pv = pg[:, 0:k * w].rearrange("p (k w) -> p k w", k=k)
                if p2_group[gi]:
                    # ACT copies everything (A|B per cell) -> bf16
                    c12t = wk.tile([128, GROUP_COLS], bf16, tag="c12",
                                   name=f"c12_{gi}")
                    c12 = c12t[:, 0:k * w]
                    nc.scalar.activation(c12, pg[:, 0:k * w], AF.Copy)
                else:
                    # ACT copies only the B-halves -> bf16
                    c1t = wk.tile([128, GROUP_COLS // 2], bf16, tag="c1",
                                  name=f"c1_{gi}")
                    c1 = c1t[:, 0:k * h]
                    nc.scalar.activation(
                        c1.rearrange("p (k h) -> p k h", k=k),
                        pv[:, :, h:], AF.Copy)
                # per-cell scans
                for kk in range(k):
                    if p2_group[gi]:
                        d0 = c12[:, kk * w:kk * w + h]
                        d1 = c12[:, kk * w + h:(kk + 1) * w]
                    else:
                        d0 = pg[:, kk * w:kk * w + h]
                        d1 = c1[:, kk * h:(kk + 1) * h]
                    # stride-0 output: every running-min write lands on the
                    # same column, so the final write IS the cell min
                    nc.vector.tensor_tensor_scan(
                        out=mins[:, j0 + kk:j0 + kk + 1].broadcast_to(
                            [128, h]),
                        data0=d0, data1=d1, initial=INF,
                        op0=ALU.min, op1=ALU.min)
                slots_done = j0 + k

                if (mins_sent == 0 and slots_done >= (5 * NSLOT) // 8
                        and gi < ngroups - 1):
                    mins_sent = slots_done
                    nc.sync.dma_start(mins_d[:, 0:mins_sent],
                                      mins[:, 0:mins_sent])
            nc.sync.dma_start(mins_d[:, mins_sent:], mins[:, mins_sent:])

    nc.compile()
    return nc


def _get_program(spec):
    if spec not in _CACHE:
        _CACHE[spec] = _build_program(spec)
    return _CACHE[spec]


# ----------------------------------------------------------------- interface

def kernel(verts, anchors, choir, hand_contacts, bps_mean, bps_scalar,
           bps_basis, _trace=False):
    from concourse.bass_utils import run_bass_kernel_spmd

    hand_contacts = np.asarray(hand_contacts, np.float32)
    spec, in_maps, metas, extra = _pack(
        verts, anchors, choir, hand_contacts, bps_mean, bps_scalar,
        bps_basis)
    _, rsel = extra
    nc = _get_program(spec)
    res = run_bass_kernel_spmd(nc, in_maps, list(range(NCORES)))

    d_sel = np.sqrt(rsel.astype(np.float64))
    anc_d = np.asarray(choir, np.float32)[:, :, 4]
    choir_loss = ((d_sel - anc_d) ** 2).mean()

    contact_sum = 0.0
    for c in range(NCORES):
        mins = np.asarray(res.results[c]["mins"], np.float64)  # (128, NSLOT)
        pids, bidx = metas[c]
        m = np.maximum(mins, 1e-12)
        cont = np.exp(-100.0 * m)                              # (128, NSLOT)
        hc = hand_contacts[bidx[None, :], pids.T]              # (128, NSLOT)
        contact_sum += ((hc - cont) ** 2).sum()

    out = (np.float32(choir_loss),
           np.float32(contact_sum / (B * P)))
    if _trace:
        return out, res
    return out


# revision 8
# speedup vs baseline: 1.0026x; 1.0026x over previous
"""DualHOILoss Trainium2 kernel, v2: candidate-pruned distance mins.

Strategy (8 NeuronCores, the 512 point-cells dealt across cores):
  - Host splits each batch's 4096 points into 32 spatial cells of 128
    (KD median splits).  For each cell it computes a provably sufficient
    vert candidate list: verts within min(U_cell, D_cut) of the cell's
    AABB, where U_cell is an upper bound on any cell point's nearest-vert
    distance (via a 64-cluster decomposition of the verts) and D_cut is
    where exp(-100 d^2) is negligible vs the 2e-2 tolerance.  This cuts
    the candidate columns from 778/cell to ~210 avg.  All 512 cells are
    dealt snake-wise by width across the 8 cores so the shared SPMD
    program's rank-max width profile has minimal padding.
  - Device computes d^2[point, cand] via K=5 bf16 matmuls into PSUM
    (coefficient vectors packed on host; matmul outputs chunked on the
    512-col grid so they never cross a PSUM bank).  Each cell is
    min-reduced by ONE DVE tensor_tensor_scan consuming two streams per
    cycle; per drain group either the scan reads the PSUM A-half
    directly while ACT copies only the B-half (P1, ACT-cheap), or ACT
    copies the whole group and the scan is all-SBUF (P2, DVE-cheap) --
    chosen per group to balance the two engines.  Each scan writes
    through a stride-0 (broadcast) output AP, so its final running-min
    write lands the cell min directly in the mins slab, which is DMA'd
    out in two chunks.
  - Host finishes both losses from host-side O(P) data: the choir loss
    directly from the packed selected-anchor distances, the contact
    loss from the [128, 64] device min slab (exp + mean).  All O(P*V)
    work stays on device; the host does only O(P+V) packing and O(P)
    finishing.

The program is compiled per width-profile (deterministic for a given
input) and cached; unseen inputs just trigger a recompile.
"""

import numpy as np

B, P, A, V = 16, 4096, 32, 778
NCORES = 8
BPC = B // NCORES
NCELL = 32          # cells per batch
NSLOT = 2 * NCELL   # slots per core
D_CUT = 0.135
GROUP_COLS = 1024   # psum f32 cols per drain group (2 banks)
BIGD = 1.0e30       # sentinel distance^2 for padding columns
INF = 3.0e38
DMA_FRAC = 0.84     # fraction of B-half copies done by the DMA engines

_CACHE = {}


# ----------------------------------------------------------------- host side

def _kd_split(pts, idx, ncell):
    if ncell == 1:
        return [idx]
    d = np.argmax(pts[idx].max(0) - pts[idx].min(0))
    order = idx[np.argsort(pts[idx][:, d], kind="stable")]
    h = len(order) // 2
    return _kd_split(pts, order[:h], ncell // 2) + _kd_split(
        pts, order[h:], ncell // 2)


def _candidates(xb, yb):
    """Per-cell (point_ids[128], cand vert ids) for one batch."""
    cells = _kd_split(xb, np.arange(P), NCELL)
    vc = _kd_split(yb, np.arange(V), 64)
    cen = np.stack([yb[c].mean(0) for c in vc])
    rad = np.array([np.linalg.norm(yb[c] - cen[i], axis=1).max()
                    for i, c in enumerate(vc)])
    d2c = np.linalg.norm(xb[:, None, :] - cen[None], axis=2)
    ubp = (d2c + rad[None]).min(1)          # per-point nearest-vert UB
    out = []
    for ci in cells:
        lo = xb[ci].min(0)
        hi = xb[ci].max(0)
        dmin = np.linalg.norm(
            np.maximum(np.maximum(lo - yb, yb - hi), 0), axis=1)
        u_box = np.linalg.norm(
            np.maximum(np.abs(lo - yb), np.abs(hi - yb)), axis=1).min()
        u = min(ubp[ci].max(), u_box, D_CUT)
        cand = np.where(dmin <= u + 1e-6)[0]
        out.append((ci, cand))
    return out


def _level_dp(widths):
    """Partition sorted-desc widths into segments padded to the segment
    max, trading padding columns against per-group fixed costs."""
    n = len(widths)
    group_tax = 350  # columns-equivalent of one drain group's fixed cost
    best = [None] * (n + 1)
    best[0] = (0.0, 0)
    for i in range(1, n + 1):
        for j in range(max(0, i - 24), i):
            wmax = widths[j]           # sorted desc: max of [j, i)
            ln = i - j
            kpg = max(1, GROUP_COLS // wmax)
            cost = (best[j][0] + wmax * ln
                    + group_tax * int(np.ceil(ln / kpg)))
            if best[i] is None or cost < best[i][0]:
                best[i] = (cost, j)
    # recover segmentation
    out = [0] * n
    i = n
    while i > 0:
        j = best[i][1]
        for t in range(j, i):
            out[t] = widths[j]
        i = j
    return out


def _plan_groups(widths):
    groups = []  # (w, k, slot0)
    j = 0
    while j < len(widths):
        w = widths[j]
        k = 1
        while (j + k < len(widths) and widths[j + k] == w
               and (k + 1) * w <= GROUP_COLS):
            k += 1
        groups.append((w, k, j))
        j += k
    return groups


def _layout(widths, groups):
    """Per-group contiguous coef layout: [rhs slots..., lhsT slots...] per
    group, so DMA prefixes cover whole groups."""
    rhs_off = [0] * len(widths)
    lhs_off = [0] * len(widths)
    goff = []
    off = 0
    for (w, k, j0) in groups:
        goff.append(off)
        for kk in range(k):
            rhs_off[j0 + kk] = off
            off += w
        for kk in range(k):
            lhs_off[j0 + kk] = off
            off += 128
    goff.append(off)
    return rhs_off, lhs_off, goff


def _pack(verts, anchors, choir, hand_contacts, bps_mean, bps_scalar,
          bps_basis):
    import ml_dtypes
    verts = np.asarray(verts, np.float32)
    anchors = np.asarray(anchors, np.float32)
    choir = np.asarray(choir, np.float32)
    hand_contacts = np.asarray(hand_contacts, np.float32)
    bps_mean = np.asarray(bps_mean, np.float32).reshape(B, 3)
    s = np.float32(np.asarray(bps_scalar).reshape(()))
    basis = np.asarray(bps_basis, np.float32).reshape(P, 3)

    u = basis[None] + choir[:, :, 1:4]                 # (B,P,3)
    x = u / s
    w = verts - bps_mean[:, None, :]                   # (B,V,3)
    uu2 = (u * u).sum(-1) / (s * s)                    # (B,P)

    idx = choir[:, :, 5].astype(np.int64)
    asel = np.take_along_axis(anchors, idx[:, :, None], axis=1)
    wsel = asel - bps_mean[:, None, :]
    rsel = np.maximum((wsel * wsel).sum(-1)
                      - (np.float32(2.0) / s) * (u * wsel).sum(-1) + uu2,
                      np.float32(1.0e-12))             # (B,P)

    rhs_all = np.empty((B, 5, V), np.float32)
    rhs_all[:, 0:3] = (w * (np.float32(-2.0) / s)).transpose(0, 2, 1)
    rhs_all[:, 3] = (w * w).sum(-1)
    rhs_all[:, 4] = 1.0
    dummy_col = np.array([0.0, 0.0, 0.0, BIGD, 1.0], np.float32)

    cells = [_candidates(x[b], w[b]) for b in range(B)]
    bwidth = []
    for b in range(B):
        tw = sum(max(64, int(np.ceil(max(len(c), 1) / 32) * 32))
                 for (_, c) in cells[b])
        bwidth.append(tw)

    # global cell->core assignment: deal all B*NCELL cells in width order,
    # snake-wise across cores, so per-rank widths are nearly equal and the
    # shared (rank-max) program profile has minimal padding
    all_cells = []
    for b in range(B):
        for ci, (pid, cand) in enumerate(cells[b]):
            wd = max(32, int(np.ceil(max(len(cand), 1) / 16) * 16))
            all_cells.append((wd, b, ci))
    all_cells.sort(key=lambda t: -t[0])
    core_slots = [[] for _ in range(NCORES)]
    for r, (wd, b, ci) in enumerate(all_cells):
        row, col = divmod(r, NCORES)
        c = col if row % 2 == 0 else NCORES - 1 - col
        core_slots[c].append((b, ci, wd))

    widths = [max(core_slots[c][j][2] for c in range(NCORES))
              for j in range(NSLOT)]
    widths = _level_dp(widths)
    # rotate the last (smallest) group's slots to the front: a tiny first
    # group primes the pipeline right after the first DMA chunk lands
    g0 = _plan_groups(widths)
    nlast = NSLOT - g0[-1][2]
    jcut = NSLOT - min(4, nlast)
    perm = list(range(jcut, NSLOT)) + list(range(0, jcut))
    widths = [widths[p] for p in perm]
    for c in range(NCORES):
        core_slots[c] = [core_slots[c][p] for p in perm]
    groups = _plan_groups(widths)
    rhs_off, lhs_off, goff = _layout(widths, groups)
    lc = goff[-1]
    spec = (tuple(widths), tuple(groups))

    in_maps = []
    metas = []
    for c in range(NCORES):
        coef = np.zeros((5, lc), np.float32)
        pids = np.empty((NSLOT, 128), np.int64)
        bidx = np.empty(NSLOT, np.int64)
        for j in range(NSLOT):
            b, ci, _ = core_slots[c][j]
            pid, cand = cells[b][ci]
            wd = widths[j]
            off = rhs_off[j]
            if len(cand) > 0:
                cp = np.concatenate(
                    [cand, np.repeat(cand[:1], wd - len(cand))])
                coef[:, off:off + wd] = rhs_all[b][:, cp]
                if len(cand) < wd:
                    coef[:, off + len(cand):off + wd] = dummy_col[:, None]
            else:
                coef[:, off:off + wd] = dummy_col[:, None]
            lt = lhs_off[j]
            coef[0:3, lt:lt + 128] = u[b][pid].T
            coef[3, lt:lt + 128] = 1.0
            coef[4, lt:lt + 128] = uu2[b][pid]
            pids[j] = pid
            bidx[j] = b
        in_maps.append({
            "coef": coef.astype(ml_dtypes.bfloat16),
        })
        metas.append((pids, bidx))
    return spec, in_maps, metas, (lc, rsel)


# --------------------------------------------------------------- device side

def _build_program(spec):
    import concourse.bacc as bacc
    import concourse.mybir as mybir
    from concourse import tile

    widths, groups = spec
    f32 = mybir.dt.float32
    bf16 = mybir.dt.bfloat16
    AF = mybir.ActivationFunctionType
    ALU = mybir.AluOpType

    total_w = sum(widths)
    rhs_off, lhs_off, goff = _layout(widths, list(groups))
    lc = goff[-1]
    ngroups = len(groups)

    # Per-group drain path: P1 (scan reads PSUM directly, ACT copies only
    # the B-half) vs P2 (ACT copies the whole group to SBUF, scan is
    # all-SBUF).  P1 is ACT-cheap, P2 is DVE-cheap; flip the smallest
    # groups to P2 until the two engines' modeled loads balance.
    p2_group = [False] * ngroups
    dve_ns = sum(0.521 * w * k + 125 * k for (w, k, _) in groups)
    act_ns = sum(0.475 * w * k + 143 for (w, k, _) in groups) + 1200
    for gi in range(ngroups - 1, -1, -1):
        w, k, _ = groups[gi]
        d_dve = -74 * k
        d_act = 0.475 * w * k
        if act_ns + d_act + 300 < dve_ns + d_dve:
            p2_group[gi] = True
            dve_ns += d_dve
            act_ns += d_act

    nc = bacc.Bacc(None, target_bir_lowering=False)
    coef_d = nc.dram_tensor("coef", [5, lc], bf16, kind="ExternalInput")
    mins_d = nc.dram_tensor("mins", [128, NSLOT], f32,
                            kind="ExternalOutput")

    with tile.TileContext(nc) as tc:
        with (
            tc.tile_pool(name="sb", bufs=1) as sb,
            tc.tile_pool(name="wk", bufs=3) as wk,
            tc.tile_pool(name="ps", bufs=4, space="PSUM") as ps,
        ):
            coef = sb.tile([5, lc], bf16, tag="coef", name="coef")
            # two prefix chunks over the per-group layout: the first covers
            # the prime group plus enough groups to bridge until the rest
            # of the slab lands (HWDGE serializes at ~625ns/DMA)
            cut1 = goff[min(7, ngroups)]
            nc.sync.dma_start(coef[:, 0:cut1], coef_d[:, 0:cut1])
            nc.sync.dma_start(coef[:, cut1:], coef_d[:, cut1:])

            # PE p-state warmup
            wtile = sb.tile([5, 512], bf16, tag="wtile", name="wtile")
            nc.vector.memset(wtile[:], 0.0)
            wps = ps.tile([128, GROUP_COLS], f32, tag="pg", name="wps")
            for _ in range(4):
                nc.tensor.matmul(wps[:, 0:512], wtile[:, 0:128], wtile[:],
                                 start=True, stop=True)
            mins = sb.tile([128, NSLOT], f32, tag="mins", name="mins")

            slots_done = 0
            mins_sent = 0
            for gi, (w, k, j0) in enumerate(groups):
                pg = ps.tile([128, GROUP_COLS], f32, tag="pg", name=f"pg{gi}")
                for kk in range(k):
                    so = rhs_off[j0 + kk]
                    lt = lhs_off[j0 + kk]
                    o = kk * w
                    # chunk on the global 512-col grid: a matmul output
                    # must not cross a PSUM bank boundary
                    cc0 = 0
                    while cc0 < w:
                        nxt_bank = ((o + cc0) // 512 + 1) * 512 - o
                        cc1 = min(cc0 + 512, w, nxt_bank)
                        nc.tensor.matmul(
                            pg[:, o + cc0:o + cc1],
                            coef[:, lt:lt + 128],
                            coef[:, so + cc0:so + cc1],
                            start=True, stop=True)
                        cc0 = cc1
                h = w // 2
                pv = pg[:, 0:k * w].rearrange("p (k w) -> p k w", k=k)
                if p2_group[gi]:
                    # ACT copies everything (A|B per cell) -> bf16
                    c12t = wk.tile([128, GROUP_COLS], bf16, tag="c12",
                                   name=f"c12_{gi}")
                    c12 = c12t[:, 0:k * w]
                    nc.scalar.activation(c12, pg[:, 0:k * w], AF.Copy)
                else:
                    # ACT copies only the B-halves -> bf16
                    c1t = wk.tile([128, GROUP_COLS // 2], bf16, tag="c1",
                                  name=f"c1_{gi}")
                    c1 = c1t[:, 0:k * h]
                    nc.scalar.activation(
                        c1.rearrange("p (k h) -> p k h", k=k),
                        pv[:, :, h:], AF.Copy)
                # per-cell scans
                for kk in range(k):
                    if p2_group[gi]:
                        d0 = c12[:, kk * w:kk * w + h]
                        d1 = c12[:, kk * w + h:(kk + 1) * w]
                    else:
                        d0 = pg[:, kk * w:kk * w + h]
                        d1 = c1[:, kk * h:(kk + 1) * h]
                    # stride-0 output: every running-min write lands on the
                    # same column, so the final write IS the cell min
                    nc.vector.tensor_tensor_scan(
                        out=mins[:, j0 + kk:j0 + kk + 1].broadcast_to(
                            [128, h]),
                        data0=d0, data1=d1, initial=INF,
                        op0=ALU.min, op1=ALU.min)
                slots_done = j0 + k

                if (mins_sent == 0 and slots_done >= (5 * NSLOT) // 8
                        and gi < ngroups - 1):
                    mins_sent = slots_done
                    nc.sync.dma_start(mins_d[:, 0:mins_sent],
                                      mins[:, 0:mins_sent])
            nc.sync.dma_start(mins_d[:, mins_sent:], mins[:, mins_sent:])

    nc.compile()
    return nc


def _get_program(spec):
    if spec not in _CACHE:
        _CACHE[spec] = _build_program(spec)
    return _CACHE[spec]


# ----------------------------------------------------------------- interface

def kernel(verts, anchors, choir, hand_contacts, bps_mean, bps_scalar,
           bps_basis, _trace=False):
    from concourse.bass_utils import run_bass_kernel_spmd

    hand_contacts = np.asarray(hand_contacts, np.float32)
    spec, in_maps, metas, extra = _pack(
        verts, anchors, choir, hand_contacts, bps_mean, bps_scalar,
        bps_basis)
    _, rsel = extra
    nc = _get_program(spec)
    res = run_bass_kernel_spmd(nc, in_maps, list(range(NCORES)))

    d_sel = np.sqrt(rsel.astype(np.float64))
    anc_d = np.asarray(choir, np.float32)[:, :, 4]
    choir_loss = ((d_sel - anc_d) ** 2).mean()

    contact_sum = 0.0
    for c in range(NCORES):
        mins = np.asarray(res.results[c]["mins"], np.float64)  # (128, NSLOT)
        pids, bidx = metas[c]
        m = np.maximum(mins, 1e-12)
        cont = np.exp(-100.0 * m)                              # (128, NSLOT)
        hc = hand_contacts[bidx[None, :], pids.T]              # (128, NSLOT)
        contact_sum += ((hc - cont) ** 2).sum()

    out = (np.float32(choir_loss),
           np.float32(contact_sum / (B * P)))
    if _trace:
        return out, res
    return out
